# revision 10
# baseline (speedup 1.0000x reference)
"""CubicFilter Trainium2 kernel: 4x(conv3x3s2+lrelu[+maxpool]) -> FC -> cubic basis mask -> feat+mask.

Sharding: pure data parallel, one image per NeuronCore (B=8 over 8 cores).
Per-core layout: image rows split into two halves stacked on partitions
(p = half*64 + ch) so every big DMA uses all 128 partitions.
Compute dtype fp16 (PE 1 cyc/col; end-to-end rel err ~3e-4), f32 accumulation,
f32 output.
"""
import numpy as np
from contextlib import ExitStack

import concourse.bass as bass
import concourse.tile as tile
from concourse import bacc, mybir
from concourse.bass_utils import run_bass_kernel_spmd

F32 = mybir.dt.float32
F16 = mybir.dt.float16
LRELU = mybir.ActivationFunctionType.Lrelu

C = 64          # channels
H = 256         # image size
NCORES = 8
NBLK = 8        # conv1 row blocks (16 input rows per half each)
NCHUNK = 16     # phase-B pixel chunks (8 rows per half each)

TAPS = [(1, 1), (0, 0), (0, 1), (0, 2), (1, 0), (1, 2), (2, 0), (2, 1), (2, 2)]


def _cubic_basis_np(h):
    x = np.arange(1, h + 1, dtype=np.float64) / h
    v = np.vander(x, increasing=False)
    xa, ya = v, v.T
    maps = np.stack([
        xa ** 3,
        xa ** 2 * ya,
        xa ** 2,
        np.ones_like(xa) + xa * ya ** 2,
        xa * ya,
        xa,
        ya ** 3,
        ya ** 2,
        ya,
        np.ones_like(xa),
    ], axis=0)
    return maps.astype(np.float32)


def _conv_taps(nc, psum, w_sb, inp, layer, n_oh, n_ow, row_shift=-1,
               first_skips_kh0=True, pbase=0, tile_pos=None):
    """Emit 9 shifted-tap matmuls for a 3x3 stride-2 pad-1 conv block.

    inp: SBUF AP [64, rows, cols]. For local output row `oh` and tap `kh`,
    the input row read is  row_shift + kh + 2*oh  (row_shift = 2*oh_first - 1
    - input_row_of_inp_row0). first_skips_kh0: kh=0 taps start at oh=1 (top pad).
    psum: [64, n_oh, n_ow] psum AP.
    """
    last = TAPS[-1]
    # split output rows so each matmul stays within one 512-element psum bank
    oh_chunk = max(1, 512 // n_ow)
    for (kh, kw) in TAPS:
        oh0 = 1 if (kh == 0 and first_skips_kh0) else 0
        ow0 = 1 if kw == 0 else 0
        c0 = 2 * ow0 + kw - 1
        for oh_start in range(0, n_oh, oh_chunk):
            oha = max(oh_start, oh0)
            ohb = min(oh_start + oh_chunk, n_oh)
            if ohb <= oha:
                continue
            r0 = row_shift + kh + 2 * oha
            rhs = inp[:, r0: r0 + 2 * (ohb - oha) - 1: 2, c0: c0 + 2 * (n_ow - ow0) - 1: 2]
            nc.tensor.matmul(
                psum[:, oha:ohb, ow0:n_ow],
                w_sb[pbase:pbase + 64, layer, kh * 3 + kw, :],
                rhs,
                start=(kh == 1 and kw == 1),
                stop=(kh, kw) == last,
                tile_position=tile_pos,
            )


def build(dbg=False):
    nc = bacc.Bacc("TRN2", target_bir_lowering=False, debug=False, num_devices=NCORES)

    feat_d = nc.dram_tensor("feat", [C, H, H], F16, kind="ExternalInput").ap()
    wt_d = nc.dram_tensor("wt", [C, 4, 9, C], F16, kind="ExternalInput").ap()
    b_d = nc.dram_tensor("bias", [4, C], F32, kind="ExternalInput").ap()
    fcw_d = nc.dram_tensor("fcw", [C, 640], F16, kind="ExternalInput").ap()
    fcb_d = nc.dram_tensor("fcb", [640], F32, kind="ExternalInput").ap()
    basis_d = nc.dram_tensor("basis", [10, H, H], F16, kind="ExternalInput").ap()
    out_d = nc.dram_tensor("out", [C, H, H], F32, kind="ExternalOutput").ap()
    if dbg:
        d_acts1 = nc.dram_tensor("d_acts1", [C, 64, 64], F16, kind="ExternalOutput").ap()
        d_acts2 = nc.dram_tensor("d_acts2", [C, 16, 16], F16, kind="ExternalOutput").ap()
        d_acts3 = nc.dram_tensor("d_acts3", [C, 4, 4], F16, kind="ExternalOutput").ap()
        d_r = nc.dram_tensor("d_r", [1, 640], F32, kind="ExternalOutput").ap()
        d_R = nc.dram_tensor("d_R", [42, C], F16, kind="ExternalOutput").ap()
        d_a1s = nc.dram_tensor("d_a1s", [128, 32, 64], F16, kind="ExternalOutput").ap()

    # g c h w views: DMA-paired with [128, rows, 256] SBUF tiles so that
    # partition p = g*64 + c, local h in [0,128)
    fr = feat_d.rearrange("c (g h) w -> g c h w", g=2)
    outr = out_d.rearrange("c (g h) w -> g c h w", g=2)

    with tile.TileContext(nc) as tc, ExitStack() as ctx:
        wp = ctx.enter_context(tc.tile_pool(name="wp", bufs=1))
        sp = ctx.enter_context(tc.tile_pool(name="sp", bufs=1))
        ps = ctx.enter_context(tc.tile_pool(name="ps", bufs=2, space="PSUM"))

        # ---- persistent loads ----
        w_sb = wp.tile([128, 4, 9, C], F16)
        nc.sync.dma_start(out=w_sb[0:64], in_=wt_d)
        nc.sync.dma_start(out=w_sb[64:128], in_=wt_d)
        b_sb = wp.tile([128, 4], F32)
        bT = b_d.transpose([1, 0])
        nc.sync.dma_start(out=b_sb[0:64], in_=bT)
        nc.sync.dma_start(out=b_sb[64:128], in_=bT)
        fcw_sb = wp.tile([C, 640], F16)
        nc.sync.dma_start(out=fcw_sb, in_=fcw_d)
        fcb_sb = wp.tile([1, 640], F32)
        nc.sync.dma_start(out=fcb_sb, in_=fcb_d)
        basis_sb = wp.tile([42, 128, H], F16)
        nc.sync.dma_start(out=basis_sb[0:10], in_=basis_d[:, 0:128, :])
        nc.sync.dma_start(out=basis_sb[32:42], in_=basis_d[:, 128:256, :])

        acts1s = wp.tile([128, 32, 64], F16)   # pooled conv1, split halves
        acts1 = wp.tile([C, 64, 64], F16)      # merged
        acts2 = wp.tile([C, 16, 16], F16)
        acts3 = wp.tile([C, 4, 4], F16)
        R_sb = wp.tile([42, C], F16)           # mask coefs, at bases 0 and 32

        # ================= Phase A: conv1 + pool over 8 blocks =================
        with tc.tile_pool(name="fp", bufs=3) as fp, tc.tile_pool(name="ap", bufs=3) as ap:
            for b in range(NBLK):
                ft = fp.tile([128, 17, H], F16)
                if b == 0:
                    nc.sync.dma_start(out=ft[0:64, 1:17], in_=fr[0:1, :, 0:16, :])
                    nc.sync.dma_start(out=ft[64:128, 1:17], in_=fr[1:2, :, 0:16, :])
                    nc.sync.dma_start(out=ft[64:128, 0:1], in_=fr[0:1, :, 127:128, :])
                else:
                    nc.sync.dma_start(out=ft, in_=fr[:, :, 16 * b - 1: 16 * b + 16, :])

                cv1 = ps.tile([128, 8, 128], F32, tag="ps")
                for rh in (0, 1):
                    pb = 64 * rh
                    # ft row 0 holds input row 16b-1 (per half); oh_first = 8b
                    _conv_taps(nc, cv1[pb:pb + 64], w_sb, ft[pb:pb + 64], 0,
                               8, 128, row_shift=0,
                               first_skips_kh0=(b == 0 and rh == 0),
                               pbase=pb, tile_pos=(64, 64) if rh else None)
                ab = ap.tile([128, 8, 128], F16)
                nc.scalar.activation(out=ab, in_=cv1, func=LRELU,
                                     bias=b_sb[:, 0:1], scale=1.0, alpha=0.01)
                t1 = ap.tile([128, 8, 64], F16)
                nc.vector.tensor_max(t1, ab[:, :, 0:127:2], ab[:, :, 1:128:2])
                nc.vector.tensor_max(acts1s[:, 4 * b:4 * b + 4, :],
                                     t1[:, 0:7:2, :], t1[:, 1:8:2, :])
            # merge split halves -> [64, 64, 64]
            nc.sync.dma_start(out=acts1[:, 0:32, :], in_=acts1s[0:64])
            nc.sync.dma_start(out=acts1[:, 32:64, :], in_=acts1s[64:128])

        # ================= conv2/3/4 + FC =================
        # conv2: acts1 [64,64,64] -> psum [64,32,32] in two 16-row chunks
        t2 = sp.tile([C, 32, 32], F16)
        for chunk in (0, 1):
            p2 = ps.tile([C, 16, 32], F32, tag="ps")
            if chunk == 0:
                # acts1 row 0 == input row 0; row_shift = 2*0-1-0 = -1
                _conv_taps(nc, p2, w_sb, acts1, 1, 16, 32, row_shift=-1,
                           first_skips_kh0=True)
            else:
                # view row 0 == input row 31; row_shift = 2*16-1-31 = 0
                _conv_taps(nc, p2, w_sb, acts1[:, 31:, :], 1, 16, 32, row_shift=0,
                           first_skips_kh0=False)
            nc.scalar.activation(out=t2[:, 16 * chunk:16 * chunk + 16, :], in_=p2,
                                 func=LRELU, bias=b_sb[0:64, 1:2], scale=1.0, alpha=0.01)
        t2p = sp.tile([C, 32, 16], F16)
        nc.vector.tensor_max(t2p, t2[:, :, 0:31:2], t2[:, :, 1:32:2])
        nc.vector.tensor_max(acts2, t2p[:, 0:31:2, :], t2p[:, 1:32:2, :])

        # conv3: acts2 [64,16,16] -> [64,8,8]
        p3 = ps.tile([C, 8, 8], F32, tag="ps")
        _conv_taps(nc, p3, w_sb, acts2, 2, 8, 8)
        t3 = sp.tile([C, 8, 8], F16)
        nc.scalar.activation(out=t3, in_=p3, func=LRELU,
                             bias=b_sb[0:64, 2:3], scale=1.0, alpha=0.01)
        t3p = sp.tile([C, 8, 4], F16)
        nc.vector.tensor_max(t3p, t3[:, :, 0:7:2], t3[:, :, 1:8:2])
        nc.vector.tensor_max(acts3, t3p[:, 0:7:2, :], t3p[:, 1:8:2, :])

        # conv4: acts3 [64,4,4] -> [64,2,2]
        p4 = ps.tile([C, 2, 2], F32, tag="ps")
        _conv_taps(nc, p4, w_sb, acts3, 3, 2, 2)
        t4 = sp.tile([C, 4], F32)
        nc.scalar.activation(out=t4, in_=p4.rearrange("p a b -> p (a b)"), func=LRELU,
                             bias=b_sb[0:64, 3:4], scale=1.0, alpha=0.01)
        xsum = sp.tile([C, 1], F32)
        nc.vector.tensor_reduce(out=xsum, in_=t4, axis=mybir.AxisListType.X,
                                op=mybir.AluOpType.add)
        x16 = sp.tile([C, 1], F16)
        nc.vector.tensor_copy(x16, xsum)

        # FC: R[j'] with j' = k*64+c ; mean/4 folded into fcw on host
        pR1 = ps.tile([1, 512], F32, tag="ps")
        nc.tensor.matmul(pR1, x16, fcw_sb[:, 0:512], start=True, stop=True)
        pR2 = ps.tile([1, 128], F32, tag="ps")
        nc.tensor.matmul(pR2, x16, fcw_sb[:, 512:640], start=True, stop=True)
        r_fl = sp.tile([1, 640], F32)
        nc.vector.tensor_add(r_fl[:, 0:512], pR1, fcb_sb[:, 0:512])
        nc.vector.tensor_add(r_fl[:, 512:640], pR2, fcb_sb[:, 512:640])
        r16 = sp.tile([1, 640], F16)
        nc.vector.tensor_copy(r16, r_fl)
        # scatter [1, (k c)] -> [10, 64] at partition bases 0 and 32
        nc.sync.dma_start(out=R_sb[0:10, :], in_=r16)
        nc.sync.dma_start(out=R_sb[32:42, :], in_=r16)

        if dbg:
            nc.sync.dma_start(out=d_acts1, in_=acts1)
            nc.sync.dma_start(out=d_acts2, in_=acts2)
            nc.sync.dma_start(out=d_acts3, in_=acts3)
            nc.sync.dma_start(out=d_r, in_=r_fl)
            nc.sync.dma_start(out=d_R, in_=R_sb)
            nc.sync.dma_start(out=d_a1s, in_=acts1s)

        # ================= Phase B: out = feat + R . basis =================
        with tc.tile_pool(name="bp", bufs=3) as bp:
            for cchunk in range(NCHUNK):
                h0 = 8 * cchunk
                fB = bp.tile([128, 8, H], F16)
                nc.sync.dma_start(out=fB, in_=fr[:, :, h0:h0 + 8, :])
                outt = bp.tile([128, 8, H], F32)
                for t in (0, 1):
                    msk = ps.tile([128, 4, H], F32, tag="ps")
                    rows = slice(h0 + 4 * t, h0 + 4 * t + 4)
                    for rh in (0, 1):
                        kb = 32 * rh
                        for s in (0, 1):
                            nc.tensor.matmul(
                                msk[64 * rh:64 * rh + 64, 2 * s:2 * s + 2, :],
                                R_sb[kb:kb + 10, :],
                                basis_sb[kb:kb + 10, h0 + 4 * t + 2 * s: h0 + 4 * t + 2 * s + 2, :],
                                start=True, stop=True,
                                tile_position=(32, 64) if rh else None,
                            )
                    nc.vector.tensor_add(outt[:, 4 * t:4 * t + 4, :],
                                         fB[:, 4 * t:4 * t + 4, :], msk)
                nc.sync.dma_start(out=outr[:, :, h0:h0 + 8, :], in_=outt)

    nc.compile()
    return nc


_NC = None


def _get_nc():
    global _NC
    if _NC is None:
        _NC = build()
    return _NC


def kernel(feat, w1, b1, w2, b2, w3, b3, w4, b4, fc_w, fc_b):
    feat = np.asarray(feat, dtype=np.float32)
    B = feat.shape[0]
    assert B == NCORES

    wt = np.stack(
        [np.asarray(w).transpose(1, 2, 3, 0).reshape(C, 9, C) for w in (w1, w2, w3, w4)],
        axis=1,
    ).astype(np.float16)  # [ci, 4, 9, co]
    bias = np.stack([np.asarray(b) for b in (b1, b2, b3, b4)], axis=0).astype(np.float32)
    # fcw[cc, j'] with j' = k*64+c ; fold the mean's /4 in
    fcw = (np.asarray(fc_w).reshape(C, 10, C).transpose(2, 1, 0).reshape(C, 640) / 4.0
           ).astype(np.float16)
    fcb = np.asarray(fc_b).reshape(C, 10).T.reshape(640).astype(np.float32).copy()
    basis = _cubic_basis_np(H).astype(np.float16)

    shared = {"wt": wt, "bias": bias, "fcw": fcw, "fcb": fcb, "basis": basis}
    in_maps = [
        {"feat": feat[i].astype(np.float16), **shared}
        for i in range(B)
    ]
    global _last_in_maps
    _last_in_maps = in_maps
    nc = _get_nc()
    res = run_bass_kernel_spmd(nc, in_maps, core_ids=list(range(NCORES)))
    return np.stack([res.results[i]["out"] for i in range(B)], axis=0)


# revision 11
# speedup vs baseline: 2.5719x; 2.5719x over previous
"""CubicFilter Trainium2 kernel: 4x(conv3x3s2+lrelu[+maxpool]) -> FC -> cubic basis mask -> feat+mask.

Sharding: pure data parallel, one image per NeuronCore (B=8 over 8 cores).
Per-core layout: image rows split into two halves stacked on partitions
(p = half*64 + ch) so every big DMA uses all 128 partitions.
Compute dtype fp16 (PE 1 cyc/col; end-to-end rel err ~3e-4), f32 accumulation,
f32 output.
"""
import numpy as np
from contextlib import ExitStack

import concourse.bass as bass
import concourse.tile as tile
from concourse import bacc, mybir
from concourse.bass_utils import run_bass_kernel_spmd

F32 = mybir.dt.float32
F16 = mybir.dt.float16
LRELU = mybir.ActivationFunctionType.Lrelu

C = 64          # channels
H = 256         # image size
NCORES = 8
NBLK = 8        # conv1 row blocks (16 input rows per half each)
NCHUNK = 16     # phase-B pixel chunks (8 rows per half each)

TAPS = [(1, 1), (0, 0), (0, 1), (0, 2), (1, 0), (1, 2), (2, 0), (2, 1), (2, 2)]


def _cubic_basis_np(h):
    x = np.arange(1, h + 1, dtype=np.float64) / h
    v = np.vander(x, increasing=False)
    xa, ya = v, v.T
    maps = np.stack([
        xa ** 3,
        xa ** 2 * ya,
        xa ** 2,
        np.ones_like(xa) + xa * ya ** 2,
        xa * ya,
        xa,
        ya ** 3,
        ya ** 2,
        ya,
        np.ones_like(xa),
    ], axis=0)
    return maps.astype(np.float32)


def _conv_taps(nc, psum, w_sb, inp, layer, n_oh, n_ow, row_shift=-1,
               first_skips_kh0=True, pbase=0, tile_pos=None):
    """Emit 9 shifted-tap matmuls for a 3x3 stride-2 pad-1 conv block.

    inp: SBUF AP [64, rows, cols]. For local output row `oh` and tap `kh`,
    the input row read is  row_shift + kh + 2*oh  (row_shift = 2*oh_first - 1
    - input_row_of_inp_row0). first_skips_kh0: kh=0 taps start at oh=1 (top pad).
    psum: [64, n_oh, n_ow] psum AP.
    """
    last = TAPS[-1]
    # split output rows so each matmul stays within one 512-element psum bank
    oh_chunk = max(1, 512 // n_ow)
    for (kh, kw) in TAPS:
        oh0 = 1 if (kh == 0 and first_skips_kh0) else 0
        ow0 = 1 if kw == 0 else 0
        c0 = 2 * ow0 + kw - 1
        for oh_start in range(0, n_oh, oh_chunk):
            oha = max(oh_start, oh0)
            ohb = min(oh_start + oh_chunk, n_oh)
            if ohb <= oha:
                continue
            r0 = row_shift + kh + 2 * oha
            rhs = inp[:, r0: r0 + 2 * (ohb - oha) - 1: 2, c0: c0 + 2 * (n_ow - ow0) - 1: 2]
            nc.tensor.matmul(
                psum[:, oha:ohb, ow0:n_ow],
                w_sb[pbase:pbase + 64, layer, kh * 3 + kw, :],
                rhs,
                start=(kh == 1 and kw == 1),
                stop=(kh, kw) == last,
                tile_position=tile_pos,
            )


def build(dbg=False):
    nc = bacc.Bacc("TRN2", target_bir_lowering=False, debug=False, num_devices=NCORES)

    feat_d = nc.dram_tensor("feat", [C, H, H], F16, kind="ExternalInput").ap()
    wt_d = nc.dram_tensor("wt", [C, 4, 9, C], F16, kind="ExternalInput").ap()
    b_d = nc.dram_tensor("bias", [4, C], F32, kind="ExternalInput").ap()
    fcw_d = nc.dram_tensor("fcw", [C, 640], F16, kind="ExternalInput").ap()
    fcb_d = nc.dram_tensor("fcb", [640], F32, kind="ExternalInput").ap()
    basis_d = nc.dram_tensor("basis", [10, H, H], F16, kind="ExternalInput").ap()
    out_d = nc.dram_tensor("out", [C, H, H], F32, kind="ExternalOutput").ap()
    if dbg:
        d_acts1 = nc.dram_tensor("d_acts1", [C, 64, 64], F16, kind="ExternalOutput").ap()
        d_acts2 = nc.dram_tensor("d_acts2", [C, 16, 16], F16, kind="ExternalOutput").ap()
        d_acts3 = nc.dram_tensor("d_acts3", [C, 4, 4], F16, kind="ExternalOutput").ap()
        d_r = nc.dram_tensor("d_r", [1, 640], F32, kind="ExternalOutput").ap()
        d_R = nc.dram_tensor("d_R", [42, C], F16, kind="ExternalOutput").ap()
        d_a1s = nc.dram_tensor("d_a1s", [128, 32, 64], F16, kind="ExternalOutput").ap()


    with tile.TileContext(nc) as tc, ExitStack() as ctx:
        wp = ctx.enter_context(tc.tile_pool(name="wp", bufs=1))
        sp = ctx.enter_context(tc.tile_pool(name="sp", bufs=1))
        ps = ctx.enter_context(tc.tile_pool(name="ps", bufs=2, space="PSUM"))

        # ---- persistent loads ----
        w_sb = wp.tile([128, 4, 9, C], F16)
        nc.sync.dma_start(out=w_sb[0:64], in_=wt_d)
        nc.sync.dma_start(out=w_sb[64:128], in_=wt_d)
        b_sb = wp.tile([128, 4], F32)
        bT = b_d.transpose([1, 0])
        nc.sync.dma_start(out=b_sb[0:64], in_=bT)
        nc.sync.dma_start(out=b_sb[64:128], in_=bT)
        fcw_sb = wp.tile([C, 640], F16)
        nc.sync.dma_start(out=fcw_sb, in_=fcw_d)
        fcb_sb = wp.tile([1, 640], F32)
        nc.sync.dma_start(out=fcb_sb, in_=fcb_d)
        basis_sb = wp.tile([42, 128, H], F16)
        nc.sync.dma_start(out=basis_sb[0:10], in_=basis_d[:, 0:128, :])
        nc.sync.dma_start(out=basis_sb[32:42], in_=basis_d[:, 128:256, :])

        acts1s = wp.tile([128, 32, 64], F16)   # pooled conv1, split halves
        acts1 = wp.tile([C, 64, 64], F16)      # merged
        acts2 = wp.tile([C, 16, 16], F16)
        acts3 = wp.tile([C, 4, 4], F16)
        R_sb = wp.tile([42, C], F16)           # mask coefs, at bases 0 and 32

        # ================= Phase A: conv1 + pool over 8 blocks =================
        with tc.tile_pool(name="fp", bufs=3) as fp, tc.tile_pool(name="ap", bufs=3) as ap:
            for b in range(NBLK):
                # two DMAs (one per half): DRAM-side outer dim = 64 channels so
                # the DGE spreads descriptors across all 16 SDMA engines
                ft = fp.tile([128, 17, H], F16)
                if b == 0:
                    nc.sync.dma_start(out=ft[0:64, 1:17], in_=feat_d[:, 0:16, :])
                else:
                    nc.sync.dma_start(out=ft[0:64], in_=feat_d[:, 16 * b - 1: 16 * b + 16, :])
                nc.sync.dma_start(out=ft[64:128], in_=feat_d[:, 127 + 16 * b: 144 + 16 * b, :])

                cv1 = ps.tile([128, 8, 128], F32, tag="ps")
                for rh in (0, 1):
                    pb = 64 * rh
                    # ft row 0 holds input row 16b-1 (per half); oh_first = 8b
                    _conv_taps(nc, cv1[pb:pb + 64], w_sb, ft[pb:pb + 64], 0,
                               8, 128, row_shift=0,
                               first_skips_kh0=(b == 0 and rh == 0),
                               pbase=pb, tile_pos=(64, 64) if rh else None)
                ab = ap.tile([128, 8, 128], F16)
                nc.scalar.activation(out=ab, in_=cv1, func=LRELU,
                                     bias=b_sb[:, 0:1], scale=1.0, alpha=0.01)
                t1 = ap.tile([128, 8, 64], F16)
                nc.vector.tensor_max(t1, ab[:, :, 0:127:2], ab[:, :, 1:128:2])
                nc.vector.tensor_max(acts1s[:, 4 * b:4 * b + 4, :],
                                     t1[:, 0:7:2, :], t1[:, 1:8:2, :])
            # merge split halves -> [64, 64, 64]
            nc.sync.dma_start(out=acts1[:, 0:32, :], in_=acts1s[0:64])
            nc.sync.dma_start(out=acts1[:, 32:64, :], in_=acts1s[64:128])

        # ================= conv2/3/4 + FC =================
        # conv2: acts1 [64,64,64] -> psum [64,32,32] in two 16-row chunks
        t2 = sp.tile([C, 32, 32], F16)
        for chunk in (0, 1):
            p2 = ps.tile([C, 16, 32], F32, tag="ps")
            if chunk == 0:
                # acts1 row 0 == input row 0; row_shift = 2*0-1-0 = -1
                _conv_taps(nc, p2, w_sb, acts1, 1, 16, 32, row_shift=-1,
                           first_skips_kh0=True)
            else:
                # view row 0 == input row 31; row_shift = 2*16-1-31 = 0
                _conv_taps(nc, p2, w_sb, acts1[:, 31:, :], 1, 16, 32, row_shift=0,
                           first_skips_kh0=False)
            nc.scalar.activation(out=t2[:, 16 * chunk:16 * chunk + 16, :], in_=p2,
                                 func=LRELU, bias=b_sb[0:64, 1:2], scale=1.0, alpha=0.01)
        t2p = sp.tile([C, 32, 16], F16)
        nc.vector.tensor_max(t2p, t2[:, :, 0:31:2], t2[:, :, 1:32:2])
        nc.vector.tensor_max(acts2, t2p[:, 0:31:2, :], t2p[:, 1:32:2, :])

        # conv3: acts2 [64,16,16] -> [64,8,8]
        p3 = ps.tile([C, 8, 8], F32, tag="ps")
        _conv_taps(nc, p3, w_sb, acts2, 2, 8, 8)
        t3 = sp.tile([C, 8, 8], F16)
        nc.scalar.activation(out=t3, in_=p3, func=LRELU,
                             bias=b_sb[0:64, 2:3], scale=1.0, alpha=0.01)
        t3p = sp.tile([C, 8, 4], F16)
        nc.vector.tensor_max(t3p, t3[:, :, 0:7:2], t3[:, :, 1:8:2])
        nc.vector.tensor_max(acts3, t3p[:, 0:7:2, :], t3p[:, 1:8:2, :])

        # conv4: acts3 [64,4,4] -> [64,2,2]
        p4 = ps.tile([C, 2, 2], F32, tag="ps")
        _conv_taps(nc, p4, w_sb, acts3, 3, 2, 2)
        t4 = sp.tile([C, 4], F32)
        nc.scalar.activation(out=t4, in_=p4.rearrange("p a b -> p (a b)"), func=LRELU,
                             bias=b_sb[0:64, 3:4], scale=1.0, alpha=0.01)
        xsum = sp.tile([C, 1], F32)
        nc.vector.tensor_reduce(out=xsum, in_=t4, axis=mybir.AxisListType.X,
                                op=mybir.AluOpType.add)
        x16 = sp.tile([C, 1], F16)
        nc.vector.tensor_copy(x16, xsum)

        # FC: R[j'] with j' = k*64+c ; mean/4 folded into fcw on host
        pR1 = ps.tile([1, 512], F32, tag="ps")
        nc.tensor.matmul(pR1, x16, fcw_sb[:, 0:512], start=True, stop=True)
        pR2 = ps.tile([1, 128], F32, tag="ps")
        nc.tensor.matmul(pR2, x16, fcw_sb[:, 512:640], start=True, stop=True)
        r_fl = sp.tile([1, 640], F32)
        nc.vector.tensor_add(r_fl[:, 0:512], pR1, fcb_sb[:, 0:512])
        nc.vector.tensor_add(r_fl[:, 512:640], pR2, fcb_sb[:, 512:640])
        r16 = sp.tile([1, 640], F16)
        nc.vector.tensor_copy(r16, r_fl)
        # scatter [1, (k c)] -> [10, 64] at partition bases 0 and 32
        nc.sync.dma_start(out=R_sb[0:10, :], in_=r16)
        nc.sync.dma_start(out=R_sb[32:42, :], in_=r16)

        if dbg:
            nc.sync.dma_start(out=d_acts1, in_=acts1)
            nc.sync.dma_start(out=d_acts2, in_=acts2)
            nc.sync.dma_start(out=d_acts3, in_=acts3)
            nc.sync.dma_start(out=d_r, in_=r_fl)
            nc.sync.dma_start(out=d_R, in_=R_sb)
            nc.sync.dma_start(out=d_a1s, in_=acts1s)

        # ================= Phase B: out = feat + R . basis =================
        with tc.tile_pool(name="bp", bufs=3) as bp:
            for cchunk in range(NCHUNK):
                h0 = 8 * cchunk
                fB = bp.tile([128, 8, H], F16)
                nc.sync.dma_start(out=fB[0:64], in_=feat_d[:, h0:h0 + 8, :])
                nc.sync.dma_start(out=fB[64:128], in_=feat_d[:, 128 + h0:128 + h0 + 8, :])
                outt = bp.tile([128, 8, H], F32)
                for t in (0, 1):
                    msk = ps.tile([128, 4, H], F32, tag="ps")
                    rows = slice(h0 + 4 * t, h0 + 4 * t + 4)
                    for rh in (0, 1):
                        kb = 32 * rh
                        for s in (0, 1):
                            nc.tensor.matmul(
                                msk[64 * rh:64 * rh + 64, 2 * s:2 * s + 2, :],
                                R_sb[kb:kb + 10, :],
                                basis_sb[kb:kb + 10, h0 + 4 * t + 2 * s: h0 + 4 * t + 2 * s + 2, :],
                                start=True, stop=True,
                                tile_position=(32, 64) if rh else None,
                            )
                    nc.vector.tensor_add(outt[:, 4 * t:4 * t + 4, :],
                                         fB[:, 4 * t:4 * t + 4, :], msk)
                nc.sync.dma_start(out=out_d[:, h0:h0 + 8, :], in_=outt[0:64])
                nc.sync.dma_start(out=out_d[:, 128 + h0:128 + h0 + 8, :], in_=outt[64:128])

    nc.compile()
    return nc


_NC = None


def _get_nc():
    global _NC
    if _NC is None:
        _NC = build()
    return _NC


def kernel(feat, w1, b1, w2, b2, w3, b3, w4, b4, fc_w, fc_b):
    feat = np.asarray(feat, dtype=np.float32)
    B = feat.shape[0]
    assert B == NCORES

    wt = np.stack(
        [np.asarray(w).transpose(1, 2, 3, 0).reshape(C, 9, C) for w in (w1, w2, w3, w4)],
        axis=1,
    ).astype(np.float16)  # [ci, 4, 9, co]
    bias = np.stack([np.asarray(b) for b in (b1, b2, b3, b4)], axis=0).astype(np.float32)
    # fcw[cc, j'] with j' = k*64+c ; fold the mean's /4 in
    fcw = (np.asarray(fc_w).reshape(C, 10, C).transpose(2, 1, 0).reshape(C, 640) / 4.0
           ).astype(np.float16)
    fcb = np.asarray(fc_b).reshape(C, 10).T.reshape(640).astype(np.float32).copy()
    basis = _cubic_basis_np(H).astype(np.float16)

    shared = {"wt": wt, "bias": bias, "fcw": fcw, "fcb": fcb, "basis": basis}
    in_maps = [
        {"feat": feat[i].astype(np.float16), **shared}
        for i in range(B)
    ]
    global _last_in_maps
    _last_in_maps = in_maps
    nc = _get_nc()
    res = run_bass_kernel_spmd(nc, in_maps, core_ids=list(range(NCORES)))
    return np.stack([res.results[i]["out"] for i in range(B)], axis=0)


# revision 12
# speedup vs baseline: 2.8032x; 1.0899x over previous
"""CubicFilter Trainium2 kernel: 4x(conv3x3s2+lrelu[+maxpool]) -> FC -> cubic basis mask -> feat+mask.

Sharding: pure data parallel, one image per NeuronCore (B=8 over 8 cores).
Per-core layout: image rows split into two halves stacked on partitions
(p = half*64 + ch) so every big DMA uses all 128 partitions.
Compute dtype fp16 (PE 1 cyc/col; end-to-end rel err ~3e-4), f32 accumulation,
f32 output.
"""
import numpy as np
from contextlib import ExitStack

import concourse.bass as bass
import concourse.tile as tile
from concourse import bacc, mybir
from concourse.bass_utils import run_bass_kernel_spmd

F32 = mybir.dt.float32
F16 = mybir.dt.float16
LRELU = mybir.ActivationFunctionType.Lrelu

C = 64          # channels
H = 256         # image size
NCORES = 8
NBLK = 4        # conv1 row blocks (32 input rows per half each)
NCHUNK = 8      # phase-B pixel chunks (16 rows per half each)

TAPS = [(1, 1), (0, 0), (0, 1), (0, 2), (1, 0), (1, 2), (2, 0), (2, 1), (2, 2)]


def _cubic_basis_np(h):
    x = np.arange(1, h + 1, dtype=np.float64) / h
    v = np.vander(x, increasing=False)
    xa, ya = v, v.T
    maps = np.stack([
        xa ** 3,
        xa ** 2 * ya,
        xa ** 2,
        np.ones_like(xa) + xa * ya ** 2,
        xa * ya,
        xa,
        ya ** 3,
        ya ** 2,
        ya,
        np.ones_like(xa),
    ], axis=0)
    return maps.astype(np.float32)


def _conv_taps(nc, psum, w_sb, inp, layer, n_oh, n_ow, row_shift=-1,
               first_skips_kh0=True, pbase=0, tile_pos=None):
    """Emit 9 shifted-tap matmuls for a 3x3 stride-2 pad-1 conv block.

    inp: SBUF AP [64, rows, cols]. For local output row `oh` and tap `kh`,
    the input row read is  row_shift + kh + 2*oh  (row_shift = 2*oh_first - 1
    - input_row_of_inp_row0). first_skips_kh0: kh=0 taps start at oh=1 (top pad).
    psum: [64, n_oh, n_ow] psum AP.
    """
    last = TAPS[-1]
    # split output rows so each matmul stays within one 512-element psum bank
    oh_chunk = max(1, 512 // n_ow)
    for (kh, kw) in TAPS:
        oh0 = 1 if (kh == 0 and first_skips_kh0) else 0
        ow0 = 1 if kw == 0 else 0
        c0 = 2 * ow0 + kw - 1
        for oh_start in range(0, n_oh, oh_chunk):
            oha = max(oh_start, oh0)
            ohb = min(oh_start + oh_chunk, n_oh)
            if ohb <= oha:
                continue
            r0 = row_shift + kh + 2 * oha
            rhs = inp[:, r0: r0 + 2 * (ohb - oha) - 1: 2, c0: c0 + 2 * (n_ow - ow0) - 1: 2]
            nc.tensor.matmul(
                psum[:, oha:ohb, ow0:n_ow],
                w_sb[pbase:pbase + 64, layer, kh * 3 + kw, :],
                rhs,
                start=(kh == 1 and kw == 1),
                stop=(kh, kw) == last,
                tile_position=tile_pos,
            )


def build(dbg=False):
    nc = bacc.Bacc("TRN2", target_bir_lowering=False, debug=False, num_devices=NCORES)

    feat_d = nc.dram_tensor("feat", [C, H, H], F16, kind="ExternalInput").ap()
    wt_d = nc.dram_tensor("wt", [C, 4, 9, C], F16, kind="ExternalInput").ap()
    b_d = nc.dram_tensor("bias", [4, C], F32, kind="ExternalInput").ap()
    fcw_d = nc.dram_tensor("fcw", [C, 640], F16, kind="ExternalInput").ap()
    fcb_d = nc.dram_tensor("fcb", [640], F32, kind="ExternalInput").ap()
    basis_d = nc.dram_tensor("basis", [10, H, H], F16, kind="ExternalInput").ap()
    out_d = nc.dram_tensor("out", [C, H, H], F32, kind="ExternalOutput").ap()
    if dbg:
        d_acts1 = nc.dram_tensor("d_acts1", [C, 64, 64], F16, kind="ExternalOutput").ap()
        d_acts2 = nc.dram_tensor("d_acts2", [C, 16, 16], F16, kind="ExternalOutput").ap()
        d_acts3 = nc.dram_tensor("d_acts3", [C, 4, 4], F16, kind="ExternalOutput").ap()
        d_r = nc.dram_tensor("d_r", [1, 640], F32, kind="ExternalOutput").ap()
        d_R = nc.dram_tensor("d_R", [42, C], F16, kind="ExternalOutput").ap()
        d_a1s = nc.dram_tensor("d_a1s", [128, 32, 64], F16, kind="ExternalOutput").ap()


    with tile.TileContext(nc) as tc, ExitStack() as ctx:
        wp = ctx.enter_context(tc.tile_pool(name="wp", bufs=1))
        sp = ctx.enter_context(tc.tile_pool(name="sp", bufs=1))
        ps = ctx.enter_context(tc.tile_pool(name="ps", bufs=2, space="PSUM"))

        # ---- persistent loads ----
        w_sb = wp.tile([128, 4, 9, C], F16)
        nc.sync.dma_start(out=w_sb[0:64], in_=wt_d)
        nc.sync.dma_start(out=w_sb[64:128], in_=wt_d)
        b_sb = wp.tile([128, 4], F32)
        bT = b_d.transpose([1, 0])
        nc.sync.dma_start(out=b_sb[0:64], in_=bT)
        nc.sync.dma_start(out=b_sb[64:128], in_=bT)
        fcw_sb = wp.tile([C, 640], F16)
        nc.sync.dma_start(out=fcw_sb, in_=fcw_d)
        fcb_sb = wp.tile([1, 640], F32)
        nc.sync.dma_start(out=fcb_sb, in_=fcb_d)
        basis_sb = wp.tile([42, 128, H], F16)
        nc.sync.dma_start(out=basis_sb[0:10], in_=basis_d[:, 0:128, :])
        nc.sync.dma_start(out=basis_sb[32:42], in_=basis_d[:, 128:256, :])

        acts1s = wp.tile([128, 32, 64], F16)   # pooled conv1, split halves
        acts1 = wp.tile([C, 64, 64], F16)      # merged
        acts2 = wp.tile([C, 16, 16], F16)
        acts3 = wp.tile([C, 4, 4], F16)
        R_sb = wp.tile([42, C], F16)           # mask coefs, at bases 0 and 32

        # ================= Phase A: conv1 + pool over 4 blocks =================
        with tc.tile_pool(name="fp", bufs=2) as fp, tc.tile_pool(name="ap", bufs=3) as ap:
            for b in range(NBLK):
                # two DMAs (one per half): DRAM-side outer dim = 64 channels so
                # the DGE spreads descriptors across all 16 SDMA engines
                ft = fp.tile([128, 33, H], F16)
                if b == 0:
                    nc.sync.dma_start(out=ft[0:64, 1:33], in_=feat_d[:, 0:32, :])
                else:
                    nc.sync.dma_start(out=ft[0:64], in_=feat_d[:, 32 * b - 1: 32 * b + 32, :])
                nc.sync.dma_start(out=ft[64:128], in_=feat_d[:, 127 + 32 * b: 160 + 32 * b, :])

                for s in (0, 1):
                    cv1 = ps.tile([128, 8, 128], F32, tag="ps")
                    for rh in (0, 1):
                        pb = 64 * rh
                        # ft row 0 holds input row 32b-1 (per half)
                        _conv_taps(nc, cv1[pb:pb + 64], w_sb, ft[pb:pb + 64], 0,
                                   8, 128, row_shift=16 * s,
                                   first_skips_kh0=(b == 0 and s == 0 and rh == 0),
                                   pbase=pb, tile_pos=(64, 64) if rh else None)
                    ab = ap.tile([128, 8, 128], F16)
                    nc.scalar.activation(out=ab, in_=cv1, func=LRELU,
                                         bias=b_sb[:, 0:1], scale=1.0, alpha=0.01)
                    t1 = ap.tile([128, 8, 64], F16)
                    nc.vector.tensor_max(t1, ab[:, :, 0:127:2], ab[:, :, 1:128:2])
                    nc.vector.tensor_max(acts1s[:, 8 * b + 4 * s:8 * b + 4 * s + 4, :],
                                         t1[:, 0:7:2, :], t1[:, 1:8:2, :])
            # merge split halves -> [64, 64, 64]
            nc.sync.dma_start(out=acts1[:, 0:32, :], in_=acts1s[0:64])
            nc.sync.dma_start(out=acts1[:, 32:64, :], in_=acts1s[64:128])

        # ================= conv2/3/4 + FC =================
        # conv2: acts1 [64,64,64] -> psum [64,32,32] in two 16-row chunks
        t2 = sp.tile([C, 32, 32], F16)
        for chunk in (0, 1):
            p2 = ps.tile([C, 16, 32], F32, tag="ps")
            if chunk == 0:
                # acts1 row 0 == input row 0; row_shift = 2*0-1-0 = -1
                _conv_taps(nc, p2, w_sb, acts1, 1, 16, 32, row_shift=-1,
                           first_skips_kh0=True)
            else:
                # view row 0 == input row 31; row_shift = 2*16-1-31 = 0
                _conv_taps(nc, p2, w_sb, acts1[:, 31:, :], 1, 16, 32, row_shift=0,
                           first_skips_kh0=False)
            nc.scalar.activation(out=t2[:, 16 * chunk:16 * chunk + 16, :], in_=p2,
                                 func=LRELU, bias=b_sb[0:64, 1:2], scale=1.0, alpha=0.01)
        t2p = sp.tile([C, 32, 16], F16)
        nc.vector.tensor_max(t2p, t2[:, :, 0:31:2], t2[:, :, 1:32:2])
        nc.vector.tensor_max(acts2, t2p[:, 0:31:2, :], t2p[:, 1:32:2, :])

        # conv3: acts2 [64,16,16] -> [64,8,8]
        p3 = ps.tile([C, 8, 8], F32, tag="ps")
        _conv_taps(nc, p3, w_sb, acts2, 2, 8, 8)
        t3 = sp.tile([C, 8, 8], F16)
        nc.scalar.activation(out=t3, in_=p3, func=LRELU,
                             bias=b_sb[0:64, 2:3], scale=1.0, alpha=0.01)
        t3p = sp.tile([C, 8, 4], F16)
        nc.vector.tensor_max(t3p, t3[:, :, 0:7:2], t3[:, :, 1:8:2])
        nc.vector.tensor_max(acts3, t3p[:, 0:7:2, :], t3p[:, 1:8:2, :])

        # conv4: acts3 [64,4,4] -> [64,2,2]
        p4 = ps.tile([C, 2, 2], F32, tag="ps")
        _conv_taps(nc, p4, w_sb, acts3, 3, 2, 2)
        t4 = sp.tile([C, 4], F32)
        nc.scalar.activation(out=t4, in_=p4.rearrange("p a b -> p (a b)"), func=LRELU,
                             bias=b_sb[0:64, 3:4], scale=1.0, alpha=0.01)
        xsum = sp.tile([C, 1], F32)
        nc.vector.tensor_reduce(out=xsum, in_=t4, axis=mybir.AxisListType.X,
                                op=mybir.AluOpType.add)
        x16 = sp.tile([C, 1], F16)
        nc.vector.tensor_copy(x16, xsum)

        # FC: R[j'] with j' = k*64+c ; mean/4 folded into fcw on host
        pR1 = ps.tile([1, 512], F32, tag="ps")
        nc.tensor.matmul(pR1, x16, fcw_sb[:, 0:512], start=True, stop=True)
        pR2 = ps.tile([1, 128], F32, tag="ps")
        nc.tensor.matmul(pR2, x16, fcw_sb[:, 512:640], start=True, stop=True)
        r_fl = sp.tile([1, 640], F32)
        nc.vector.tensor_add(r_fl[:, 0:512], pR1, fcb_sb[:, 0:512])
        nc.vector.tensor_add(r_fl[:, 512:640], pR2, fcb_sb[:, 512:640])
        r16 = sp.tile([1, 640], F16)
        nc.vector.tensor_copy(r16, r_fl)
        # scatter [1, (k c)] -> [10, 64] at partition bases 0 and 32
        nc.sync.dma_start(out=R_sb[0:10, :], in_=r16)
        nc.sync.dma_start(out=R_sb[32:42, :], in_=r16)

        if dbg:
            nc.sync.dma_start(out=d_acts1, in_=acts1)
            nc.sync.dma_start(out=d_acts2, in_=acts2)
            nc.sync.dma_start(out=d_acts3, in_=acts3)
            nc.sync.dma_start(out=d_r, in_=r_fl)
            nc.sync.dma_start(out=d_R, in_=R_sb)
            nc.sync.dma_start(out=d_a1s, in_=acts1s)

        # ================= Phase B: out = feat + R . basis =================
        with tc.tile_pool(name="bp", bufs=2) as bp:
            for cchunk in range(NCHUNK):
                h0 = 16 * cchunk
                fB = bp.tile([128, 16, H], F16)
                nc.sync.dma_start(out=fB[0:64], in_=feat_d[:, h0:h0 + 16, :])
                nc.sync.dma_start(out=fB[64:128], in_=feat_d[:, 128 + h0:128 + h0 + 16, :])
                outt = bp.tile([128, 16, H], F32)
                for t in range(4):
                    msk = ps.tile([128, 4, H], F32, tag="ps")
                    for rh in (0, 1):
                        kb = 32 * rh
                        for s in (0, 1):
                            r0 = h0 + 4 * t + 2 * s
                            nc.tensor.matmul(
                                msk[64 * rh:64 * rh + 64, 2 * s:2 * s + 2, :],
                                R_sb[kb:kb + 10, :],
                                basis_sb[kb:kb + 10, r0: r0 + 2, :],
                                start=True, stop=True,
                                tile_position=(32, 64) if rh else None,
                            )
                    nc.vector.tensor_add(outt[:, 4 * t:4 * t + 4, :],
                                         fB[:, 4 * t:4 * t + 4, :], msk)
                # outputs go out on the second HWDGE ring (scalar engine)
                nc.scalar.dma_start(out=out_d[:, h0:h0 + 16, :], in_=outt[0:64])
                nc.scalar.dma_start(out=out_d[:, 128 + h0:128 + h0 + 16, :], in_=outt[64:128])

    nc.compile()
    return nc


_NC = None


def _get_nc():
    global _NC
    if _NC is None:
        _NC = build()
    return _NC


def kernel(feat, w1, b1, w2, b2, w3, b3, w4, b4, fc_w, fc_b):
    feat = np.asarray(feat, dtype=np.float32)
    B = feat.shape[0]
    assert B == NCORES

    wt = np.stack(
        [np.asarray(w).transpose(1, 2, 3, 0).reshape(C, 9, C) for w in (w1, w2, w3, w4)],
        axis=1,
    ).astype(np.float16)  # [ci, 4, 9, co]
    bias = np.stack([np.asarray(b) for b in (b1, b2, b3, b4)], axis=0).astype(np.float32)
    # fcw[cc, j'] with j' = k*64+c ; fold the mean's /4 in
    fcw = (np.asarray(fc_w).reshape(C, 10, C).transpose(2, 1, 0).reshape(C, 640) / 4.0
           ).astype(np.float16)
    fcb = np.asarray(fc_b).reshape(C, 10).T.reshape(640).astype(np.float32).copy()
    basis = _cubic_basis_np(H).astype(np.float16)

    shared = {"wt": wt, "bias": bias, "fcw": fcw, "fcb": fcb, "basis": basis}
    in_maps = [
        {"feat": feat[i].astype(np.float16), **shared}
        for i in range(B)
    ]
    global _last_in_maps
    _last_in_maps = in_maps
    nc = _get_nc()
    res = run_bass_kernel_spmd(nc, in_maps, core_ids=list(range(NCORES)))
    return np.stack([res.results[i]["out"] for i in range(B)], axis=0)


# revision 14
# speedup vs baseline: 3.9494x; 1.4089x over previous
"""CubicFilter Trainium2 kernel: 4x(conv3x3s2+lrelu[+maxpool]) -> FC -> cubic basis mask -> feat+mask.

Sharding: pure data parallel, one image per NeuronCore (B=8 over 8 cores).
Per-core layout: image rows split into two halves stacked on partitions
(p = half*64 + ch) so every big DMA uses all 128 partitions.
Compute dtype fp16 (PE 1 cyc/col; end-to-end rel err ~3e-4), f32 accumulation,
f32 output.
"""
import numpy as np
from contextlib import ExitStack

import concourse.bass as bass
import concourse.tile as tile
from concourse import bacc, mybir
from concourse.bass_utils import run_bass_kernel_spmd

F32 = mybir.dt.float32
F16 = mybir.dt.float16
LRELU = mybir.ActivationFunctionType.Lrelu

C = 64          # channels
H = 256         # image size
NCORES = 8
NBLK = 4        # conv1 row blocks (32 input rows per half each)
NCHUNK = 8      # phase-B pixel chunks (16 rows per half each)

TAPS = [(1, 1), (0, 0), (0, 1), (0, 2), (1, 0), (1, 2), (2, 0), (2, 1), (2, 2)]


def _cubic_basis_np(h):
    x = np.arange(1, h + 1, dtype=np.float64) / h
    v = np.vander(x, increasing=False)
    xa, ya = v, v.T
    maps = np.stack([
        xa ** 3,
        xa ** 2 * ya,
        xa ** 2,
        np.ones_like(xa) + xa * ya ** 2,
        xa * ya,
        xa,
        ya ** 3,
        ya ** 2,
        ya,
        np.ones_like(xa),
    ], axis=0)
    return maps.astype(np.float32)


def _conv_taps(nc, lanes, w_sb, layer, n_oh, n_ow, row_shift=-1,
               skip_kh0_lane=None):
    """Emit 9 shifted-tap matmuls for a 3x3 stride-2 pad-1 conv block.

    lanes: list of (inp_ap [64,rows,cols], psum_ap [64,n_oh,n_ow], pbase, tile_pos)
    emitted interleaved so disjoint PE quadrants overlap.
    For local output row `oh` and tap `kh`, the input row read is
    row_shift + kh + 2*oh. skip_kh0_lane: lane index whose kh=0 taps
    start at oh=1 (top image padding), or "all".
    """
    last = TAPS[-1]
    # split output rows so each matmul stays within one 512-element psum bank
    oh_chunk = max(1, 512 // n_ow)
    for (kh, kw) in TAPS:
        ow0 = 1 if kw == 0 else 0
        c0 = 2 * ow0 + kw - 1
        for oh_start in range(0, n_oh, oh_chunk):
            for li, (inp, psum, pbase, tile_pos) in enumerate(lanes):
                skip = (skip_kh0_lane == "all") or (li == skip_kh0_lane)
                oh0 = 1 if (kh == 0 and skip) else 0
                oha = max(oh_start, oh0)
                ohb = min(oh_start + oh_chunk, n_oh)
                if ohb <= oha:
                    continue
                r0 = row_shift + kh + 2 * oha
                rhs = inp[:, r0: r0 + 2 * (ohb - oha) - 1: 2,
                          c0: c0 + 2 * (n_ow - ow0) - 1: 2]
                nc.tensor.matmul(
                    psum[:, oha:ohb, ow0:n_ow],
                    w_sb[pbase:pbase + 64, layer, kh * 3 + kw, :],
                    rhs,
                    start=(kh == 1 and kw == 1),
                    stop=(kh, kw) == last,
                    tile_position=tile_pos,
                )


def build(dbg=False):
    nc = bacc.Bacc("TRN2", target_bir_lowering=False, debug=False, num_devices=NCORES)

    feat_d = nc.dram_tensor("feat", [C, H, H], F16, kind="ExternalInput").ap()
    wt_d = nc.dram_tensor("wt", [C, 4, 9, C], F16, kind="ExternalInput").ap()
    b_d = nc.dram_tensor("bias", [4, C], F32, kind="ExternalInput").ap()
    fcw_d = nc.dram_tensor("fcw", [C, 640], F16, kind="ExternalInput").ap()
    fcb_d = nc.dram_tensor("fcb", [640], F32, kind="ExternalInput").ap()
    basis_d = nc.dram_tensor("basis", [10, H, H], F16, kind="ExternalInput").ap()
    out_d = nc.dram_tensor("out", [C, H, H], F32, kind="ExternalOutput").ap()
    if dbg:
        d_acts1 = nc.dram_tensor("d_acts1", [C, 64, 64], F16, kind="ExternalOutput").ap()
        d_acts2 = nc.dram_tensor("d_acts2", [C, 16, 16], F16, kind="ExternalOutput").ap()
        d_acts3 = nc.dram_tensor("d_acts3", [C, 4, 4], F16, kind="ExternalOutput").ap()
        d_r = nc.dram_tensor("d_r", [1, 640], F32, kind="ExternalOutput").ap()
        d_R = nc.dram_tensor("d_R", [42, C], F16, kind="ExternalOutput").ap()
        d_a1s = nc.dram_tensor("d_a1s", [128, 32, 64], F16, kind="ExternalOutput").ap()


    with tile.TileContext(nc) as tc, ExitStack() as ctx:
        wp = ctx.enter_context(tc.tile_pool(name="wp", bufs=1))
        sp = ctx.enter_context(tc.tile_pool(name="sp", bufs=1))
        ps = ctx.enter_context(tc.tile_pool(name="ps", bufs=2, space="PSUM"))

        # ---- persistent loads ----
        w_sb = wp.tile([128, 4, 9, C], F16)
        nc.sync.dma_start(out=w_sb[0:64], in_=wt_d)
        nc.sync.dma_start(out=w_sb[64:128], in_=wt_d)
        b_sb = wp.tile([128, 4], F32)
        bT = b_d.transpose([1, 0])
        nc.sync.dma_start(out=b_sb[0:64], in_=bT)
        nc.sync.dma_start(out=b_sb[64:128], in_=bT)
        fcw_sb = wp.tile([C, 640], F16)
        nc.sync.dma_start(out=fcw_sb, in_=fcw_d)
        fcb_sb = wp.tile([1, 640], F32)
        nc.sync.dma_start(out=fcb_sb, in_=fcb_d)

        acts1s = wp.tile([128, 32, 64], F16)   # pooled conv1, split halves
        acts1 = wp.tile([C, 64, 64], F16)      # merged
        acts2 = wp.tile([C, 16, 16], F16)
        acts3 = wp.tile([C, 4, 4], F16)
        R_sb = wp.tile([42, C], F16)           # mask coefs, at bases 0 and 32

        # ================= Phase A: feat resident + conv1 over 8 chunks ===========
        # feat_sb row r of half g holds global input row (g*128 + r - 1);
        # i.e. one extra leading row per half so both halves index uniformly
        # (h1's row 0 = global 127, needed by its first conv rows).
        feat_sb = wp.tile([128, 129, H], F16)
        for k in range(4):
            nc.sync.dma_start(out=feat_sb[0:64, 1 + 32 * k: 33 + 32 * k],
                              in_=feat_d[:, 32 * k: 32 * k + 32, :])
        h1r = [0, 33, 65, 97, 129]
        for k in range(4):
            nc.sync.dma_start(out=feat_sb[64:128, h1r[k]:h1r[k + 1]],
                              in_=feat_d[:, 127 + h1r[k]: 127 + h1r[k + 1], :])

        with tc.tile_pool(name="ap", bufs=3) as ap:
            for s in range(8):
                cv1 = ps.tile([128, 8, 128], F32, tag="ps")
                lanes = [
                    (feat_sb[0:64], cv1[0:64], 0, None),
                    (feat_sb[64:128], cv1[64:128], 64, (64, 64)),
                ]
                _conv_taps(nc, lanes, w_sb, 0, 8, 128, row_shift=16 * s,
                           skip_kh0_lane=0 if s == 0 else None)
                ab = ap.tile([128, 8, 128], F16)
                nc.scalar.activation(out=ab, in_=cv1, func=LRELU,
                                     bias=b_sb[:, 0:1], scale=1.0, alpha=0.01)
                t1 = ap.tile([128, 8, 64], F16)
                nc.vector.tensor_max(t1, ab[:, :, 0:127:2], ab[:, :, 1:128:2])
                nc.vector.tensor_max(acts1s[:, 4 * s:4 * s + 4, :],
                                     t1[:, 0:7:2, :], t1[:, 1:8:2, :])
            # merge split halves -> [64, 64, 64]
            nc.sync.dma_start(out=acts1[:, 0:32, :], in_=acts1s[0:64])
            nc.sync.dma_start(out=acts1[:, 32:64, :], in_=acts1s[64:128])

        # ================= conv2/3/4 + FC =================
        # conv2: acts1 [64,64,64] -> psum [64,32,32] in two 16-row chunks
        t2 = sp.tile([C, 32, 32], F16)
        for chunk in (0, 1):
            p2 = ps.tile([C, 16, 32], F32, tag="ps")
            if chunk == 0:
                # acts1 row 0 == input row 0; row_shift = 2*0-1-0 = -1
                _conv_taps(nc, [(acts1, p2, 0, None)], w_sb, 1, 16, 32,
                           row_shift=-1, skip_kh0_lane="all")
            else:
                # view row 0 == input row 31; row_shift = 2*16-1-31 = 0
                _conv_taps(nc, [(acts1[:, 31:, :], p2, 0, None)], w_sb, 1, 16, 32,
                           row_shift=0)
            nc.scalar.activation(out=t2[:, 16 * chunk:16 * chunk + 16, :], in_=p2,
                                 func=LRELU, bias=b_sb[0:64, 1:2], scale=1.0, alpha=0.01)
        t2p = sp.tile([C, 32, 16], F16)
        nc.vector.tensor_max(t2p, t2[:, :, 0:31:2], t2[:, :, 1:32:2])
        nc.vector.tensor_max(acts2, t2p[:, 0:31:2, :], t2p[:, 1:32:2, :])

        # conv3: acts2 [64,16,16] -> [64,8,8]
        p3 = ps.tile([C, 8, 8], F32, tag="ps")
        _conv_taps(nc, [(acts2, p3, 0, None)], w_sb, 2, 8, 8, skip_kh0_lane="all")
        t3 = sp.tile([C, 8, 8], F16)
        nc.scalar.activation(out=t3, in_=p3, func=LRELU,
                             bias=b_sb[0:64, 2:3], scale=1.0, alpha=0.01)
        t3p = sp.tile([C, 8, 4], F16)
        nc.vector.tensor_max(t3p, t3[:, :, 0:7:2], t3[:, :, 1:8:2])
        nc.vector.tensor_max(acts3, t3p[:, 0:7:2, :], t3p[:, 1:8:2, :])

        # conv4: acts3 [64,4,4] -> [64,2,2]
        p4 = ps.tile([C, 2, 2], F32, tag="ps")
        _conv_taps(nc, [(acts3, p4, 0, None)], w_sb, 3, 2, 2, skip_kh0_lane="all")
        t4 = sp.tile([C, 4], F32)
        nc.scalar.activation(out=t4, in_=p4.rearrange("p a b -> p (a b)"), func=LRELU,
                             bias=b_sb[0:64, 3:4], scale=1.0, alpha=0.01)
        xsum = sp.tile([C, 1], F32)
        nc.vector.tensor_reduce(out=xsum, in_=t4, axis=mybir.AxisListType.X,
                                op=mybir.AluOpType.add)
        x16 = sp.tile([C, 1], F16)
        nc.vector.tensor_copy(x16, xsum)

        # FC: R[j'] with j' = k*64+c ; mean/4 folded into fcw on host
        pR1 = ps.tile([1, 512], F32, tag="ps")
        nc.tensor.matmul(pR1, x16, fcw_sb[:, 0:512], start=True, stop=True)
        pR2 = ps.tile([1, 128], F32, tag="ps")
        nc.tensor.matmul(pR2, x16, fcw_sb[:, 512:640], start=True, stop=True)
        r_fl = sp.tile([1, 640], F32)
        nc.vector.tensor_add(r_fl[:, 0:512], pR1, fcb_sb[:, 0:512])
        nc.vector.tensor_add(r_fl[:, 512:640], pR2, fcb_sb[:, 512:640])
        r16 = sp.tile([1, 640], F16)
        nc.vector.tensor_copy(r16, r_fl)
        # scatter [1, (k c)] -> [10, 64] at partition bases 0 and 32
        nc.sync.dma_start(out=R_sb[0:10, :], in_=r16)
        nc.sync.dma_start(out=R_sb[32:42, :], in_=r16)

        if dbg:
            nc.sync.dma_start(out=d_acts1, in_=acts1)
            nc.sync.dma_start(out=d_acts2, in_=acts2)
            nc.sync.dma_start(out=d_acts3, in_=acts3)
            nc.sync.dma_start(out=d_r, in_=r_fl)
            nc.sync.dma_start(out=d_R, in_=R_sb)
            nc.sync.dma_start(out=d_a1s, in_=acts1s)

        # ================= Phase B: out = feat + R . basis =================
        with tc.tile_pool(name="bp", bufs=2) as bp:
            for cchunk in range(NCHUNK):
                h0 = 16 * cchunk
                bas = bp.tile([42, 16, H], F16)
                nc.sync.dma_start(out=bas[0:10], in_=basis_d[:, h0:h0 + 16, :])
                nc.sync.dma_start(out=bas[32:42], in_=basis_d[:, 128 + h0:128 + h0 + 16, :])
                outt = bp.tile([128, 16, H], F32)
                for t in range(4):
                    msk = ps.tile([128, 4, H], F32, tag="ps")
                    for s in (0, 1):
                        for rh in (0, 1):
                            kb = 32 * rh
                            r0 = 4 * t + 2 * s
                            nc.tensor.matmul(
                                msk[64 * rh:64 * rh + 64, 2 * s:2 * s + 2, :],
                                R_sb[kb:kb + 10, :],
                                bas[kb:kb + 10, r0: r0 + 2, :],
                                start=True, stop=True,
                                tile_position=(32, 64) if rh else None,
                            )
                    nc.vector.tensor_add(outt[:, 4 * t:4 * t + 4, :],
                                         feat_sb[:, 1 + h0 + 4 * t: 1 + h0 + 4 * t + 4, :],
                                         msk)
                # outputs go out on the second HWDGE ring (scalar engine)
                nc.scalar.dma_start(out=out_d[:, h0:h0 + 16, :], in_=outt[0:64])
                nc.scalar.dma_start(out=out_d[:, 128 + h0:128 + h0 + 16, :], in_=outt[64:128])

    nc.compile()
    return nc


_NC = None


def _get_nc():
    global _NC
    if _NC is None:
        _NC = build()
    return _NC


def kernel(feat, w1, b1, w2, b2, w3, b3, w4, b4, fc_w, fc_b):
    feat = np.asarray(feat, dtype=np.float32)
    B = feat.shape[0]
    assert B == NCORES

    wt = np.stack(
        [np.asarray(w).transpose(1, 2, 3, 0).reshape(C, 9, C) for w in (w1, w2, w3, w4)],
        axis=1,
    ).astype(np.float16)  # [ci, 4, 9, co]
    bias = np.stack([np.asarray(b) for b in (b1, b2, b3, b4)], axis=0).astype(np.float32)
    # fcw[cc, j'] with j' = k*64+c ; fold the mean's /4 in
    fcw = (np.asarray(fc_w).reshape(C, 10, C).transpose(2, 1, 0).reshape(C, 640) / 4.0
           ).astype(np.float16)
    fcb = np.asarray(fc_b).reshape(C, 10).T.reshape(640).astype(np.float32).copy()
    basis = _cubic_basis_np(H).astype(np.float16)

    shared = {"wt": wt, "bias": bias, "fcw": fcw, "fcb": fcb, "basis": basis}
    in_maps = [
        {"feat": feat[i].astype(np.float16), **shared}
        for i in range(B)
    ]
    global _last_in_maps
    _last_in_maps = in_maps
    nc = _get_nc()
    res = run_bass_kernel_spmd(nc, in_maps, core_ids=list(range(NCORES)))
    return np.stack([res.results[i]["out"] for i in range(B)], axis=0)
